# revision 1
# baseline (speedup 1.0000x reference)
"""Causal attention (B=4, L=4096, D=2048, HD=128) on 8 TRN2 NeuronCores.

Sharding: 8 cores = 4 batches x 2 fold-halves. Core c handles batch b=c//2
and query blocks {i, 3-i} (1024 rows each) where i=c%2 — the "fold" split
balances causal attention work exactly across the two cores of a batch.
Each core recomputes K/V for all 4096 keys of its batch (no collectives).

The on-device program is identical on all cores (SPMD); per-core behavior
comes only from the data: a block-permuted transposed input xT and two
slot-bias vectors that enable/disable the two fold-dependent key blocks
(bias 0 keeps scores, bias -50 drives exp() to ~1e-22, i.e. masks).

Matmuls run in float32r (single-pass FP22 multiply, fp32 accumulate); the
post-softmax AV stage runs bf16 (exp output is cast for free on the ACT
engine) because fp32r matmuls with free dim < 256 drop to 1/4 rate.

Phases are interleaved (projections for the key blocks phase A needs ->
attention A -> remaining projections -> attention B, with out-projection
emitted per phase) so ACT exp and DVE copies overlap PE matmul work.

Layouts (partition dim first):
  xT      [D=2048, 4096]   x[b].T with key blocks permuted to local order
  Qt, Kt  [HD=128, Lq/Lk]  projections, head dim on partitions
  V_aug   [k, HD+2] bf16   natural V with a ones column -> fused row-sums
  scores  [k=128, q=512]   one matmul per tile; exp+slot-bias on ACT
  AV out  [q=128, 130]     col 128 = softmax denominator; fp32 PSUM
  outT    [D, q]           final projection, transposed; host adds bo and
                           transposes back
"""

import numpy as np
import ml_dtypes

B, L, D, HD = 4, 4096, 2048, 128
BLK = 1024            # fold block (4 per batch)
LQ = 2 * BLK          # queries per core
LK = L                # keys per core
ND = D // 128         # 16 d-tiles
NRB = LK // 512       # 8 column blocks for projections
NEG = -50.0           # slot-disable bias (exp(x-50) ~ 0)
MASKVAL = -30000.0    # intra-tile causal mask additive value

_cached = {}


def _build_program():
    import concourse.bass as bass
    import concourse.tile as tile
    from concourse import bacc, mybir
    from concourse.masks import make_identity

    f32 = mybir.dt.float32
    f32r = mybir.dt.float32r
    bf16 = mybir.dt.bfloat16
    nc = bacc.Bacc("TRN2", target_bir_lowering=False, debug=False)

    xT_d = nc.dram_tensor("xT", (D, LK), bf16, kind="ExternalInput")
    wq_d = nc.dram_tensor("wq", (D, HD), bf16, kind="ExternalInput")
    wk_d = nc.dram_tensor("wk", (D, HD), bf16, kind="ExternalInput")
    wv_d = nc.dram_tensor("wv", (D, HD), bf16, kind="ExternalInput")
    wo_d = nc.dram_tensor("wo", (HD, D), bf16, kind="ExternalInput")
    bias_d = nc.dram_tensor("biases", (128, 8), f32, kind="ExternalInput")
    out_d = nc.dram_tensor("outT", (D, LQ), f32, kind="ExternalOutput")
    rs_d = nc.dram_tensor("rowsums", (1, LQ), f32, kind="ExternalOutput")

    # phase -> list of (local_kblk, kind); kind in {"diag", "full", "bA", "bB"}
    SLOTS = {
        0: [(0, "diag"), (2, "bA")],
        1: [(0, "full"), (1, "diag"), (2, "full"), (3, "bB")],
    }
    # rb visit order: phase A needs key blocks 0 and 2 (rb 0,1,4,5) and
    # Qt cols [0,1024) (rb 0,1); B needs the rest.
    RB_ORDER = [0, 1, 4, 5, 2, 3, 6, 7]

    with tile.TileContext(nc) as tc:
        with (
            tc.tile_pool(name="const", bufs=1) as cpool,
            tc.tile_pool(name="xt", bufs=5) as xtpool,
            tc.tile_pool(name="vt", bufs=3) as vtpool,
            tc.tile_pool(name="expst", bufs=12) as epool,
            tc.tile_pool(name="outsb", bufs=6) as outpool,
            tc.tile_pool(name="psum", bufs=1, space="PSUM") as psum,
        ):
            # ---- persistent SBUF tensors ----
            wq_s = cpool.tile([128, ND, 128], bf16, tag="wq")
            wk_s = cpool.tile([128, ND, 128], bf16, tag="wk")
            wv_s = cpool.tile([128, ND, 128], bf16, tag="wv")
            wo_s = cpool.tile([128, D], bf16, tag="wo")
            bias_s = cpool.tile([128, 8], f32, tag="biases")
            kt_s = cpool.tile([128, LK], bf16, tag="kt")
            qt_s = cpool.tile([128, LQ], bf16, tag="qt")
            v_s = cpool.tile([128, LK], bf16, tag="v")
            ones_s = cpool.tile([128, 1], bf16, tag="ones")
            rs_s = cpool.tile([1, LQ], f32, tag="rs")
            masks_s = cpool.tile([128, 4 * 512], f32, tag="masks")
            ot_s = cpool.tile([128, LQ], bf16, tag="ot")
            identb_s = cpool.tile([128, 128], bf16, tag="identb")

            xT_r = xT_d.ap().rearrange("(n p) m -> p n m", p=128)

            # first xt block + wk first so PE can start ASAP
            xts = {}
            rb0 = RB_ORDER[0]
            nc.sync.dma_start(
                wk_s[:], wk_d.ap().rearrange("(n p) m -> p n m", p=128)
            )
            xts[rb0] = xtpool.tile([128, ND, 512], bf16, tag="xt", name="xt")
            for ch in range(2):
                nc.sync.dma_start(
                    xts[rb0][:, ch * 4:(ch + 1) * 4, :],
                    xT_r[:, ch * 4:(ch + 1) * 4, rb0 * 512:(rb0 + 1) * 512],
                )
            nc.sync.dma_start(
                wv_s[:], wv_d.ap().rearrange("(n p) m -> p n m", p=128)
            )
            nc.sync.dma_start(
                wq_s[:], wq_d.ap().rearrange("(n p) m -> p n m", p=128)
            )
            for ch in range(2, 4):
                nc.sync.dma_start(
                    xts[rb0][:, ch * 4:(ch + 1) * 4, :],
                    xT_r[:, ch * 4:(ch + 1) * 4, rb0 * 512:(rb0 + 1) * 512],
                )
            nc.sync.dma_start(wo_s[:], wo_d.ap())
            nc.sync.dma_start(bias_s[:], bias_d.ap())

            make_identity(nc, identb_s[:])
            nc.gpsimd.memset(ones_s[:], 1.0)
            # 4 causal mask tiles for relative offsets delta = 0,128,256,384:
            # keep 0 where q_free >= k_part + delta, else MASKVAL
            nc.gpsimd.memset(masks_s[:], 0.0)
            for m in range(4):
                nc.gpsimd.affine_select(
                    out=masks_s[:, m * 512:(m + 1) * 512],
                    in_=masks_s[:, m * 512:(m + 1) * 512],
                    compare_op=mybir.AluOpType.is_ge,
                    fill=MASKVAL,
                    base=-(m * 128),
                    channel_multiplier=-1,
                    pattern=[[1, 512]],
                )

            bq_ap = bias_s[:, 0:1]
            bk_ap = bias_s[:, 1:2]
            bv_ap = bias_s[:, 2:3]
            slot_bias = {"bA": bias_s[:, 3:4], "bB": bias_s[:, 4:5]}

            def emit_rb(rb, prefetch_rb=None):
                """Projections for one 512-wide column block of xT."""
                xt = xts.pop(rb)
                if prefetch_rb is not None:
                    xts[prefetch_rb] = xtpool.tile(
                        [128, ND, 512], bf16, tag="xt", name="xt"
                    )
                    for ch in range(4):
                        nc.sync.dma_start(
                            xts[prefetch_rb][:, ch * 4:(ch + 1) * 4, :],
                            xT_r[:, ch * 4:(ch + 1) * 4,
                                 prefetch_rb * 512:(prefetch_rb + 1) * 512],
                        )
                cs = slice(rb * 512, (rb + 1) * 512)

                pk = psum.tile([128, 512], f32, tag="acc512", bufs=2, name="pk")
                for dt in range(ND):
                    nc.tensor.matmul(
                        pk[:], wk_s[:, dt, :], xt[:, dt, :],
                        start=(dt == 0), stop=(dt == ND - 1),
                    )
                nc.vector.tensor_scalar_add(kt_s[:, cs], pk[:], bk_ap)

                pv = psum.tile([128, 512], f32, tag="acc512", bufs=2, name="pv")
                for dt in range(ND):
                    nc.tensor.matmul(
                        pv[:], wv_s[:, dt, :], xt[:, dt, :],
                        start=(dt == 0), stop=(dt == ND - 1),
                    )
                vt_tmp = vtpool.tile([128, 512], bf16, tag="vt_tmp")
                nc.vector.tensor_scalar_add(vt_tmp[:], pv[:], bv_ap)
                for s in range(4):
                    ktile = rb * 4 + s
                    vp = psum.tile([128, 128], bf16, tag="acc512", bufs=2,
                                   name="vp")
                    nc.tensor.transpose(
                        vp[:], vt_tmp[:, s * 128:(s + 1) * 128], identb_s[:]
                    )
                    nc.vector.tensor_copy(
                        v_s[:, ktile * 128:(ktile + 1) * 128], vp[:]
                    )

                if rb < LQ // 512:
                    pq = psum.tile(
                        [128, 512], f32, tag="acc512", bufs=2, name="pq"
                    )
                    for dt in range(ND):
                        nc.tensor.matmul(
                            pq[:], wq_s[:, dt, :], xt[:, dt, :],
                            start=(dt == 0), stop=(dt == ND - 1),
                        )
                    nc.vector.tensor_scalar_add(qt_s[:, cs], pq[:], bq_ap)

            def emit_attn_u(phase, u, filler=None):
                q0 = phase * BLK + u * 512
                klist = []  # (ktile_global, mask_idx or None, bias_key)
                for kblk, kind in SLOTS[phase]:
                    for t in range(8):
                        if kind == "diag":
                            drel = t * 128 - u * 512
                            if drel >= 512:
                                continue
                            midx = drel // 128 if drel >= 0 else None
                            klist.append((kblk * 8 + t, midx, None))
                        else:
                            bkey = kind if kind in slot_bias else None
                            klist.append((kblk * 8 + t, None, bkey))

                ot_acc = psum.tile([128, 512], f32, tag="otacc", bufs=2,
                                   name="ot_acc")
                rs_acc = psum.tile([1, 512], f32, tag="rs", bufs=1,
                                   name="rs_acc")
                n = len(klist)
                sts = [None] * n
                ests = [None] * n

                def emit_st(ki):
                    kt, midx, bkey = klist[ki]
                    st = psum.tile([128, 512], f32, tag="st", bufs=3,
                                   name="st")
                    nc.tensor.matmul(
                        st[:],
                        kt_s[:, kt * 128:(kt + 1) * 128],
                        qt_s[:, q0:q0 + 512],
                        start=True, stop=True,
                    )
                    if midx is not None:
                        nc.vector.tensor_add(
                            st[:], st[:],
                            masks_s[:, midx * 512:(midx + 1) * 512],
                        )
                    est = epool.tile([128, 512], bf16, tag="est")
                    nc.scalar.activation(
                        est[:], st[:],
                        mybir.ActivationFunctionType.Exp,
                        bias=slot_bias[bkey] if bkey else 0.0,
                    )
                    ests[ki] = est

                ngroups = (n + 3) // 4
                group_est = []
                pi = 0
                emit_st(0)
                if n > 1:
                    emit_st(1)
                for ki in range(n):
                    kt, midx, bkey = klist[ki]
                    if ki + 2 < n:
                        emit_st(ki + 2)
                    if filler is not None and ki % 2 == 1:
                        next(filler, None)
                    first, last = ki == 0, ki == n - 1
                    est = ests[ki]
                    nc.tensor.matmul(
                        ot_acc[:],
                        v_s[:, kt * 128:(kt + 1) * 128],
                        est[:],
                        start=first, stop=last,
                    )
                    # row-sums: tree-add groups of 4 est (pairs on gpsimd,
                    # final on DVE), one rowsum matmul per group
                    group_est.append(est)
                    if len(group_est) == 4 or last:
                        g = group_est
                        if len(g) == 1:
                            rs_rhs = g[0]
                        elif len(g) == 2:
                            esum = epool.tile([128, 512], bf16,
                                              tag="esum", name="esum")
                            nc.vector.tensor_add(esum[:], g[0][:], g[1][:])
                            rs_rhs = esum
                        else:
                            ea = epool.tile([128, 512], bf16,
                                            tag="esum", name="ea")
                            nc.gpsimd.tensor_add(ea[:], g[0][:], g[1][:])
                            if len(g) == 3:
                                eb = g[2]
                            else:
                                eb = epool.tile([128, 512], bf16,
                                                tag="esum2", name="eb")
                                nc.gpsimd.tensor_add(eb[:], g[2][:], g[3][:])
                            esum = epool.tile([128, 512], bf16,
                                              tag="esum3", name="esum")
                            nc.vector.tensor_add(esum[:], ea[:], eb[:])
                            rs_rhs = esum
                        nc.tensor.matmul(
                            rs_acc[:], ones_s[:], rs_rhs[:],
                            start=(pi == 0), stop=(pi == ngroups - 1),
                        )
                        group_est = []
                        pi += 1

                qb = phase * 2 + u
                nc.vector.tensor_copy(
                    ot_s[:, qb * 512:(qb + 1) * 512], ot_acc[:]
                )
                nc.vector.tensor_copy(
                    rs_s[:, qb * 512:(qb + 1) * 512], rs_acc[:]
                )

            def outproj_filler(qb, dts):
                for dt in dts:
                    emit_outproj(qb, [dt])
                    yield

            def emit_outproj(qb, dts):
                # out-projection chunk (unnormalized; host divides by
                # rowsums). Copies alternate DVE/ACT to split the load.
                for dt in dts:
                    po = psum.tile([128, 512], f32, tag="acc512", bufs=2,
                                   name="po")
                    nc.tensor.matmul(
                        po[:],
                        wo_s[:, dt * 128:(dt + 1) * 128],
                        ot_s[:, qb * 512:(qb + 1) * 512],
                        start=True, stop=True,
                    )
                    orow = outpool.tile([128, 512], f32, tag="orow",
                                        name="orow")
                    if dt % 3 == 2:
                        nc.scalar.activation(
                            orow[:], po[:],
                            mybir.ActivationFunctionType.Copy,
                        )
                    else:
                        nc.vector.tensor_copy(orow[:], po[:])
                    nc.sync.dma_start(
                        out_d.ap()[dt * 128:(dt + 1) * 128,
                                   qb * 512:(qb + 1) * 512],
                        orow[:],
                    )

            # ---- interleaved schedule ----
            for j in range(4):
                emit_rb(RB_ORDER[j], prefetch_rb=RB_ORDER[j + 1])
            emit_attn_u(0, 0)
            emit_rb(2, prefetch_rb=3)
            emit_outproj(0, range(0, 8))
            emit_attn_u(0, 1)
            emit_rb(3, prefetch_rb=6)
            emit_outproj(0, range(8, ND))
            emit_outproj(1, range(0, 8))
            emit_rb(6, prefetch_rb=7)
            emit_outproj(1, range(8, ND))
            emit_rb(7, prefetch_rb=None)
            emit_attn_u(1, 0)
            emit_attn_u(1, 1, filler=outproj_filler(2, range(ND)))
            emit_outproj(3, range(ND))
            nc.sync.dma_start(rs_d.ap(), rs_s[:])

    nc.compile()
    return nc


def _get_program():
    if "nc" not in _cached:
        _cached["nc"] = _build_program()
    return _cached["nc"]


def _perm_blocks(i):
    # local order [qA, qB, o1, o2]
    return [0, 3, 1, 2] if i == 0 else [1, 2, 0, 3]


def make_in_maps(x, Wq, bq, Wk, bk, Wv, bv, Wo, bo):
    scale = 1.0 / np.sqrt(np.float32(HD))
    wq_s = (Wq * scale).astype(np.float32)
    bq_s = (bq * scale).astype(np.float32)
    in_maps = []
    for c in range(8):
        i, b = c % 2, c // 2
        perm = _perm_blocks(i)
        xbT = x[b].T  # (D, L) view
        xT = np.concatenate(
            [xbT[:, p * BLK:(p + 1) * BLK] for p in perm], axis=1
        ).astype(ml_dtypes.bfloat16)
        biases = np.zeros((128, 8), np.float32)
        biases[:, 0] = bq_s
        biases[:, 1] = bk.astype(np.float32)
        biases[:, 2] = bv.astype(np.float32)
        biases[:, 3] = NEG if i == 0 else 0.0   # phase A, slot kblk=2
        biases[:, 4] = 0.0 if i == 0 else NEG   # phase B, slot kblk=3
        in_maps.append({
            "xT": np.ascontiguousarray(xT),
            "wq": wq_s.astype(ml_dtypes.bfloat16),
            "wk": Wk.astype(ml_dtypes.bfloat16),
            "wv": Wv.astype(ml_dtypes.bfloat16),
            "wo": Wo.astype(ml_dtypes.bfloat16),
            "biases": biases,
        })
    return in_maps


def assemble_output(results, bo):
    out = np.empty((B, L, D), np.float32)
    for c in range(8):
        i, b = c % 2, c // 2
        perm = _perm_blocks(i)
        outT = results[c]["outT"] / results[c]["rowsums"]  # (D, LQ)
        qA, qB = perm[0], perm[1]
        out[b, qA * BLK:(qA + 1) * BLK, :] = outT[:, 0:BLK].T
        out[b, qB * BLK:(qB + 1) * BLK, :] = outT[:, BLK:2 * BLK].T
    out += bo.astype(np.float32)
    return out


def kernel(x, Wq, bq, Wk, bk, Wv, bv, Wo, bo):
    from concourse.bass_utils import run_bass_kernel_spmd

    nc = _get_program()
    in_maps = make_in_maps(
        np.asarray(x), np.asarray(Wq), np.asarray(bq), np.asarray(Wk),
        np.asarray(bk), np.asarray(Wv), np.asarray(bv), np.asarray(Wo),
        np.asarray(bo),
    )
    res = run_bass_kernel_spmd(nc, in_maps, core_ids=list(range(8)))
    return assemble_output(res.results, np.asarray(bo))



# revision 3
# speedup vs baseline: 1.0354x; 1.0354x over previous
"""Causal attention (B=4, L=4096, D=2048, HD=128) on 8 TRN2 NeuronCores.

Sharding: 8 cores = 4 batches x 2 fold-halves. Core c handles batch b=c//2
and query blocks {i, 3-i} (1024 rows each) where i=c%2 — the "fold" split
balances causal attention work exactly across the two cores of a batch.
Each core recomputes K/V for all 4096 keys of its batch (no collectives).

The on-device program is identical on all cores (SPMD); per-core behavior
comes only from the data: a block-permuted repacked input xr and two
slot-bias vectors that enable/disable the two fold-dependent key blocks
(bias 0 keeps scores, bias -50 drives exp() to ~1e-22, i.e. masks).

v2 changes vs v1:
  - host repacks x and the weights so every DMA has >=4KB contiguous
    per-partition lines (one DMA per 512-col xt tile instead of 4, one
    per weight); output is written bf16 in [128, qb, dt, 512] layout,
    two DMAs per 512-query block instead of 16.
  - score tiles are computed in PAIRS ([128,1024] PSUM spanning 2 banks)
    so one ACT exp call covers 2 k-tiles, amortizing the 352-cycle ACT
    fixed overhead (1147ns/pair vs 2x720ns).
  - the est tree-adds for row-sums moved from gpsimd to DVE on pairs.
  - attention units take a "filler" generator (projection or out-proj
    chunks) and interleave one chunk per score pair so PE never waits
    on the ACT exp pipeline.

Layouts (partition dim first):
  xr      [128, 8, 16, 512]  x[b].T block-permuted: [p, rb, dt, col]
  Qt, Kt  [HD=128, Lq/Lk]    projections, head dim on partitions
  v_s     [k, HD] slabs      natural V per 128-key tile (PE transpose)
  scores  [k=128, 1024]      two k-tiles per PSUM pair tile; exp on ACT
  outT    [128, 4, 16, 512]  bf16 [p, qb, dt, col]; host divides by
                             row-sums, transposes back, adds bo
"""

import numpy as np
import ml_dtypes

B, L, D, HD = 4, 4096, 2048, 128
BLK = 1024            # fold block (4 per batch)
LQ = 2 * BLK          # queries per core
LK = L                # keys per core
ND = D // 128         # 16 d-tiles
NRB = LK // 512       # 8 column blocks for projections
NEG = -50.0           # slot-disable bias (exp(x-50) ~ 0)
MASKVAL = -30000.0    # intra-tile causal mask additive value

_cached = {}


def _build_program():
    import concourse.bass as bass
    import concourse.tile as tile
    from concourse import bacc, mybir
    from concourse.masks import make_identity

    f32 = mybir.dt.float32
    bf16 = mybir.dt.bfloat16
    nc = bacc.Bacc("TRN2", target_bir_lowering=False, debug=False)

    xr_d = nc.dram_tensor("xr", (128, NRB, ND, 512), bf16,
                          kind="ExternalInput")
    wq_d = nc.dram_tensor("wq", (128, ND, 128), bf16, kind="ExternalInput")
    wk_d = nc.dram_tensor("wk", (128, ND, 128), bf16, kind="ExternalInput")
    wv_d = nc.dram_tensor("wv", (128, ND, 128), bf16, kind="ExternalInput")
    wo_d = nc.dram_tensor("wo", (HD, D), bf16, kind="ExternalInput")
    bias_d = nc.dram_tensor("biases", (128, 8), f32, kind="ExternalInput")
    out_d = nc.dram_tensor("outT", (128, 4, ND, 512), bf16,
                           kind="ExternalOutput")
    rs_d = nc.dram_tensor("rowsums", (1, LQ), f32, kind="ExternalOutput")

    # phase -> list of (local_kblk, kind); kind in {"diag", "full", "bA", "bB"}
    SLOTS = {
        0: [(0, "diag"), (2, "bA")],
        1: [(0, "full"), (1, "diag"), (2, "full"), (3, "bB")],
    }

    with tile.TileContext(nc) as tc:
        with (
            tc.tile_pool(name="const", bufs=1) as cpool,
            tc.tile_pool(name="xt", bufs=3) as xtpool,
            tc.tile_pool(name="vt", bufs=3) as vtpool,
            tc.tile_pool(name="expst", bufs=6) as epool,
            tc.tile_pool(name="outsb", bufs=2) as outpool,
            tc.tile_pool(name="psum", bufs=1, space="PSUM") as psum,
        ):
            # ---- persistent SBUF tensors ----
            wq_s = cpool.tile([128, ND, 128], bf16, tag="wq")
            wk_s = cpool.tile([128, ND, 128], bf16, tag="wk")
            wv_s = cpool.tile([128, ND, 128], bf16, tag="wv")
            wo_s = cpool.tile([128, D], bf16, tag="wo")
            bias_s = cpool.tile([128, 8], f32, tag="biases")
            kt_s = cpool.tile([128, LK], bf16, tag="kt")
            qt_s = cpool.tile([128, LQ], bf16, tag="qt")
            v_s = cpool.tile([128, LK], bf16, tag="v")
            ones_s = cpool.tile([128, 1], bf16, tag="ones")
            rs_s = cpool.tile([1, LQ], f32, tag="rs")
            masks_s = cpool.tile([128, 4 * 512], f32, tag="masks")
            ot_s = cpool.tile([128, LQ], bf16, tag="ot")
            identb_s = cpool.tile([128, 128], bf16, tag="identb")

            # first xt block + wk first so PE can start ASAP
            xts = {}
            nc.sync.dma_start(wk_s[:], wk_d.ap())
            xts[0] = xtpool.tile([128, ND, 512], bf16, tag="xt", name="xt")
            for ch in range(4):
                nc.sync.dma_start(
                    xts[0][:, ch * 4:(ch + 1) * 4, :],
                    xr_d.ap()[:, 0, ch * 4:(ch + 1) * 4, :],
                )
            nc.sync.dma_start(wv_s[:], wv_d.ap())
            nc.sync.dma_start(wq_s[:], wq_d.ap())
            xts[1] = xtpool.tile([128, ND, 512], bf16, tag="xt", name="xt")
            nc.sync.dma_start(xts[1][:], xr_d.ap()[:, 1])
            nc.sync.dma_start(wo_s[:], wo_d.ap())
            nc.sync.dma_start(bias_s[:], bias_d.ap())

            make_identity(nc, identb_s[:])
            nc.gpsimd.memset(ones_s[:], 1.0)
            # 4 causal mask tiles for relative offsets delta = 0,128,256,384:
            # keep 0 where q_free >= k_part + delta, else MASKVAL
            nc.gpsimd.memset(masks_s[:], 0.0)
            for m in range(4):
                nc.gpsimd.affine_select(
                    out=masks_s[:, m * 512:(m + 1) * 512],
                    in_=masks_s[:, m * 512:(m + 1) * 512],
                    compare_op=mybir.AluOpType.is_ge,
                    fill=MASKVAL,
                    base=-(m * 128),
                    channel_multiplier=-1,
                    pattern=[[1, 512]],
                )

            bq_ap = bias_s[:, 0:1]
            bk_ap = bias_s[:, 1:2]
            bv_ap = bias_s[:, 2:3]
            slot_bias = {"bA": bias_s[:, 3:4], "bB": bias_s[:, 4:5]}

            def prefetch(rb):
                xts[rb] = xtpool.tile([128, ND, 512], bf16, tag="xt",
                                      name="xt")
                nc.sync.dma_start(xts[rb][:], xr_d.ap()[:, rb])

            def emit_rb_gen(rb, prefetch_rb=None):
                """Projections for one 512-wide column block of xr.
                Yields between ~1us chunks so it can fill attention gaps."""
                xt = xts.pop(rb)
                if prefetch_rb is not None:
                    prefetch(prefetch_rb)
                cs = slice(rb * 512, (rb + 1) * 512)

                pk = psum.tile([128, 512], f32, tag="acc512", bufs=2,
                               name="pk")
                for dt in range(ND):
                    nc.tensor.matmul(
                        pk[:], wk_s[:, dt, :], xt[:, dt, :],
                        start=(dt == 0), stop=(dt == ND - 1),
                    )
                    if dt % 4 == 3:
                        yield
                nc.vector.tensor_scalar_add(kt_s[:, cs], pk[:], bk_ap)

                pv = psum.tile([128, 512], f32, tag="acc512", bufs=2,
                               name="pv")
                for dt in range(ND):
                    nc.tensor.matmul(
                        pv[:], wv_s[:, dt, :], xt[:, dt, :],
                        start=(dt == 0), stop=(dt == ND - 1),
                    )
                    if dt % 4 == 3:
                        yield
                vt_tmp = vtpool.tile([128, 512], bf16, tag="vt_tmp")
                nc.vector.tensor_scalar_add(vt_tmp[:], pv[:], bv_ap)
                for s in range(4):
                    ktile = rb * 4 + s
                    vp = psum.tile([128, 128], bf16, tag="acc512", bufs=2,
                                   name="vp")
                    nc.tensor.transpose(
                        vp[:], vt_tmp[:, s * 128:(s + 1) * 128], identb_s[:]
                    )
                    nc.vector.tensor_copy(
                        v_s[:, ktile * 128:(ktile + 1) * 128], vp[:]
                    )
                yield

                if rb < LQ // 512:
                    pq = psum.tile([128, 512], f32, tag="acc512", bufs=2,
                                   name="pq")
                    for dt in range(ND):
                        nc.tensor.matmul(
                            pq[:], wq_s[:, dt, :], xt[:, dt, :],
                            start=(dt == 0), stop=(dt == ND - 1),
                        )
                        if dt % 4 == 3:
                            yield
                    nc.vector.tensor_scalar_add(qt_s[:, cs], pq[:], bq_ap)

            def emit_rb(rb, prefetch_rb=None):
                for _ in emit_rb_gen(rb, prefetch_rb):
                    pass

            def build_pairs(phase, u):
                """Pairs of k-tiles sharing one exp: (kt_a, kt_b, mask_off,
                bkey). mask_off indexes masks_s[:, off:off+1024]."""
                pairs = []
                for kblk, kind in SLOTS[phase]:
                    tiles = []
                    for t in range(8):
                        if kind == "diag":
                            drel = t * 128 - u * 512
                            if drel >= 512:
                                continue
                            midx = drel // 128 if drel >= 0 else None
                            tiles.append((kblk * 8 + t, midx))
                        else:
                            tiles.append((kblk * 8 + t, None))
                    bkey = kind if kind in slot_bias else None
                    # tiles with masks come in runs of consecutive midx
                    i = 0
                    while i < len(tiles):
                        (ta, ma), (tb, mb) = tiles[i], tiles[i + 1]
                        assert (ma is None) == (mb is None)
                        moff = None if ma is None else ma * 512
                        pairs.append((ta, tb, moff, bkey))
                        i += 2
                return pairs

            def emit_attn_u(phase, u, filler=None):
                q0 = phase * BLK + u * 512
                pairs = build_pairs(phase, u)
                n = len(pairs)
                ngroups = n // 2
                ot_acc = psum.tile([128, 512], f32, tag="otacc", bufs=1,
                                   name="ot_acc")
                rs_acc = psum.tile([1, 512], f32, tag="rs", bufs=1,
                                   name="rs_acc")
                ests = [None] * n

                def emit_pair(pi):
                    ta, tb, moff, bkey = pairs[pi]
                    stp = psum.tile([128, 1024], f32, tag="stp", bufs=2,
                                    name="stp")
                    nc.tensor.matmul(
                        stp[:, 0:512],
                        kt_s[:, ta * 128:(ta + 1) * 128],
                        qt_s[:, q0:q0 + 512],
                        start=True, stop=True,
                    )
                    nc.tensor.matmul(
                        stp[:, 512:1024],
                        kt_s[:, tb * 128:(tb + 1) * 128],
                        qt_s[:, q0:q0 + 512],
                        start=True, stop=True,
                    )
                    if moff is not None:
                        nc.vector.tensor_add(
                            stp[:], stp[:], masks_s[:, moff:moff + 1024]
                        )
                    est = epool.tile([128, 1024], bf16, tag="est")
                    nc.scalar.activation(
                        est[:], stp[:],
                        mybir.ActivationFunctionType.Exp,
                        bias=slot_bias[bkey] if bkey else 0.0,
                    )
                    ests[pi] = est

                emit_pair(0)
                if n > 1:
                    emit_pair(1)
                for pi in range(n):
                    ta, tb, moff, bkey = pairs[pi]
                    if pi + 2 < n:
                        emit_pair(pi + 2)
                    if filler is not None:
                        next(filler, None)
                    est = ests[pi]
                    nc.tensor.matmul(
                        ot_acc[:],
                        v_s[:, ta * 128:(ta + 1) * 128],
                        est[:, 0:512],
                        start=(pi == 0), stop=False,
                    )
                    nc.tensor.matmul(
                        ot_acc[:],
                        v_s[:, tb * 128:(tb + 1) * 128],
                        est[:, 512:1024],
                        start=False, stop=(pi == n - 1),
                    )
                    if pi % 2 == 1:
                        g = pi // 2
                        esum = epool.tile([128, 1024], bf16, tag="esum",
                                          name="esum")
                        nc.vector.tensor_add(
                            esum[:], ests[pi - 1][:], est[:]
                        )
                        fold = epool.tile([128, 512], bf16, tag="fold",
                                          name="fold")
                        nc.vector.tensor_add(
                            fold[:], esum[:, 0:512], esum[:, 512:1024]
                        )
                        nc.tensor.matmul(
                            rs_acc[:], ones_s[:], fold[:],
                            start=(g == 0), stop=(g == ngroups - 1),
                        )

                qb = phase * 2 + u
                nc.vector.tensor_copy(
                    ot_s[:, qb * 512:(qb + 1) * 512], ot_acc[:]
                )
                nc.vector.tensor_copy(
                    rs_s[:, qb * 512:(qb + 1) * 512], rs_acc[:]
                )
                if filler is not None:
                    for _ in filler:  # drain unconsumed filler chunks
                        pass

            def outproj_gen(qb, on_act=False):
                """Out-projection for one 512-query block into a bf16 slab,
                two DMAs (dt 0-7, 8-15). Yields per dt chunk."""
                slab = outpool.tile([128, ND, 512], bf16, tag="oslab",
                                    name="oslab")
                for dt in range(ND):
                    po = psum.tile([128, 512], f32, tag="acc512", bufs=2,
                                   name="po")
                    nc.tensor.matmul(
                        po[:],
                        wo_s[:, dt * 128:(dt + 1) * 128],
                        ot_s[:, qb * 512:(qb + 1) * 512],
                        start=True, stop=True,
                    )
                    if on_act and dt % 3 == 2:
                        nc.scalar.activation(
                            slab[:, dt, :], po[:],
                            mybir.ActivationFunctionType.Copy,
                        )
                    else:
                        nc.vector.tensor_copy(slab[:, dt, :], po[:])
                    if dt == ND // 2 - 1:
                        nc.sync.dma_start(
                            out_d.ap()[:, qb, 0:ND // 2], slab[:, 0:ND // 2]
                        )
                    elif dt == ND - 1:
                        nc.sync.dma_start(
                            out_d.ap()[:, qb, ND // 2:ND],
                            slab[:, ND // 2:ND],
                        )
                    yield

            def emit_outproj(qb, on_act=False):
                for _ in outproj_gen(qb, on_act):
                    pass

            def chain(*gens):
                for g in gens:
                    for x in g:
                        yield x

            # ---- interleaved schedule ----
            # phase 0 needs local kblks 0 (rbs 0,1) and 2 (rbs 4,5) plus
            # Qt[0:1024) (rbs 0,1); phase 1 needs everything.
            emit_rb(0, prefetch_rb=4)
            emit_rb(1, prefetch_rb=5)
            emit_rb(4, prefetch_rb=2)
            emit_rb(5, prefetch_rb=3)
            emit_attn_u(0, 0, filler=emit_rb_gen(2, prefetch_rb=6))
            emit_attn_u(0, 1, filler=emit_rb_gen(3, prefetch_rb=7))
            emit_rb(6)
            emit_rb(7)
            emit_outproj(0, on_act=True)
            emit_attn_u(1, 0, filler=outproj_gen(1))
            emit_attn_u(1, 1, filler=outproj_gen(2))
            emit_outproj(3, on_act=True)
            nc.sync.dma_start(rs_d.ap(), rs_s[:])

    nc.compile()
    return nc


def _get_program():
    if "nc" not in _cached:
        _cached["nc"] = _build_program()
    return _cached["nc"]


def _perm_blocks(i):
    # local order [qA, qB, o1, o2]
    return [0, 3, 1, 2] if i == 0 else [1, 2, 0, 3]


def _repack_w(w):
    # (D, HD) -> [128, ND, 128] with per-partition contiguous lines
    return np.ascontiguousarray(
        w.reshape(ND, 128, HD).transpose(1, 0, 2)
    ).astype(ml_dtypes.bfloat16)


def make_in_maps(x, Wq, bq, Wk, bk, Wv, bv, Wo, bo):
    scale = 1.0 / np.sqrt(np.float32(HD))
    wq_r = _repack_w((Wq * scale).astype(np.float32))
    wk_r = _repack_w(Wk.astype(np.float32))
    wv_r = _repack_w(Wv.astype(np.float32))
    bq_s = (bq * scale).astype(np.float32)
    in_maps = []
    for c in range(8):
        i, b = c % 2, c // 2
        perm = _perm_blocks(i)
        xbT = x[b].T  # (D, L) view
        xT = np.concatenate(
            [xbT[:, p * BLK:(p + 1) * BLK] for p in perm], axis=1
        )
        # (D, L) -> [128, NRB, ND, 512]: xr[p, rb, dt, c] = xT[dt*128+p,
        # rb*512+c]
        xr = np.ascontiguousarray(
            xT.reshape(ND, 128, NRB, 512).transpose(1, 2, 0, 3)
        ).astype(ml_dtypes.bfloat16)
        biases = np.zeros((128, 8), np.float32)
        biases[:, 0] = bq_s
        biases[:, 1] = bk.astype(np.float32)
        biases[:, 2] = bv.astype(np.float32)
        biases[:, 3] = NEG if i == 0 else 0.0   # phase A, slot kblk=2
        biases[:, 4] = 0.0 if i == 0 else NEG   # phase B, slot kblk=3
        in_maps.append({
            "xr": xr,
            "wq": wq_r,
            "wk": wk_r,
            "wv": wv_r,
            "wo": Wo.astype(ml_dtypes.bfloat16),
            "biases": biases,
        })
    return in_maps


def assemble_output(results, bo):
    out = np.empty((B, L, D), np.float32)
    for c in range(8):
        i, b = c % 2, c // 2
        perm = _perm_blocks(i)
        arr = np.asarray(results[c]["outT"], dtype=np.float32)
        # [128, 4, ND, 512] -> (D, LQ)
        outT = arr.transpose(2, 0, 1, 3).reshape(D, LQ)
        outT /= np.asarray(results[c]["rowsums"], dtype=np.float32)
        qA, qB = perm[0], perm[1]
        out[b, qA * BLK:(qA + 1) * BLK, :] = outT[:, 0:BLK].T
        out[b, qB * BLK:(qB + 1) * BLK, :] = outT[:, BLK:2 * BLK].T
    out += bo.astype(np.float32)
    return out


def kernel(x, Wq, bq, Wk, bk, Wv, bv, Wo, bo):
    from concourse.bass_utils import run_bass_kernel_spmd

    nc = _get_program()
    in_maps = make_in_maps(
        np.asarray(x), np.asarray(Wq), np.asarray(bq), np.asarray(Wk),
        np.asarray(bk), np.asarray(Wv), np.asarray(bv), np.asarray(Wo),
        np.asarray(bo),
    )
    res = run_bass_kernel_spmd(nc, in_maps, core_ids=list(range(8)))
    return assemble_output(res.results, np.asarray(bo))


# revision 9
# speedup vs baseline: 1.0699x; 1.0333x over previous
"""Causal attention (B=4, L=4096, D=2048, HD=128) on 8 TRN2 NeuronCores.

Sharding: 8 cores = 4 batches x 2 fold-halves. Core c handles batch b=c//2
and query blocks {i, 3-i} (1024 rows each) where i=c%2 — the "fold" split
balances causal attention work exactly across the two cores of a batch.
Each core recomputes K/V for all 4096 keys of its batch (no collectives).

The on-device program is identical on all cores (SPMD); per-core behavior
comes only from the data: a block-permuted repacked input xr and two
slot-bias vectors that enable/disable the two fold-dependent key blocks
(bias 0 keeps scores, bias -50 drives exp() to ~1e-22, i.e. masks).

v2 changes vs v1:
  - host repacks x and the weights so every DMA has >=4KB contiguous
    per-partition lines (one DMA per 512-col xt tile instead of 4, one
    per weight); output is written bf16 in [128, qb, dt, 512] layout,
    two DMAs per 512-query block instead of 16.
  - score tiles are computed in PAIRS ([128,1024] PSUM spanning 2 banks)
    so one ACT exp call covers 2 k-tiles, amortizing the 352-cycle ACT
    fixed overhead (1147ns/pair vs 2x720ns).
  - the est tree-adds for row-sums moved from gpsimd to DVE on pairs.
  - attention units take a "filler" generator (projection or out-proj
    chunks) and interleave one chunk per score pair so PE never waits
    on the ACT exp pipeline.

Layouts (partition dim first):
  xr      [128, 8, 16, 512]  x[b].T block-permuted: [p, rb, dt, col]
  Qt, Kt  [HD=128, Lq/Lk]    projections, head dim on partitions
  v_s     [k, HD] slabs      natural V per 128-key tile (PE transpose)
  scores  [k=128, 1024]      two k-tiles per PSUM pair tile; exp on ACT
  outT    [128, 4, 16, 512]  bf16 [p, qb, dt, col]; host divides by
                             row-sums, transposes back, adds bo
"""

import numpy as np
import ml_dtypes

B, L, D, HD = 4, 4096, 2048, 128
BLK = 1024            # fold block (4 per batch)
LQ = 2 * BLK          # queries per core
LK = L                # keys per core
ND = D // 128         # 16 d-tiles
NRB = LK // 512       # 8 column blocks for projections
NEG = -50.0           # slot-disable bias (exp(x-50) ~ 0)
MASKVAL = -30000.0    # intra-tile causal mask additive value

_cached = {}


def _build_program():
    import concourse.bass as bass
    import concourse.tile as tile
    from concourse import bacc, mybir
    from concourse.masks import make_identity

    f32 = mybir.dt.float32
    bf16 = mybir.dt.bfloat16
    nc = bacc.Bacc("TRN2", target_bir_lowering=False, debug=False)

    xr_d = nc.dram_tensor("xr", (128, NRB, ND, 512), bf16,
                          kind="ExternalInput")
    wq_d = nc.dram_tensor("wq", (128, ND, 128), bf16, kind="ExternalInput")
    wk_d = nc.dram_tensor("wk", (128, ND, 128), bf16, kind="ExternalInput")
    wv_d = nc.dram_tensor("wv", (128, ND, 128), bf16, kind="ExternalInput")
    wo_d = nc.dram_tensor("wo", (HD, D), bf16, kind="ExternalInput")
    bias_d = nc.dram_tensor("biases", (128, 8), f32, kind="ExternalInput")
    out_d = nc.dram_tensor("outT", (128, 4, ND, 512), bf16,
                           kind="ExternalOutput")
    rs_d = nc.dram_tensor("rowsums", (1, LQ), f32, kind="ExternalOutput")

    # phase -> list of (local_kblk, kind); kind in {"diag", "full", "bA", "bB"}
    SLOTS = {
        0: [(0, "diag"), (2, "bA")],
        1: [(0, "full"), (1, "diag"), (2, "full"), (3, "bB")],
    }

    with tile.TileContext(nc) as tc:
        with (
            tc.tile_pool(name="const", bufs=1) as cpool,
            tc.tile_pool(name="xt", bufs=3) as xtpool,
            tc.tile_pool(name="vt", bufs=3) as vtpool,
            tc.tile_pool(name="expst", bufs=6) as epool,
            tc.tile_pool(name="outsb", bufs=2) as outpool,
            tc.tile_pool(name="psum", bufs=1, space="PSUM") as psum,
        ):
            # ---- persistent SBUF tensors ----
            wq_s = cpool.tile([128, ND, 128], bf16, tag="wq")
            wk_s = cpool.tile([128, ND, 128], bf16, tag="wk")
            wv_s = cpool.tile([128, ND, 128], bf16, tag="wv")
            wo_s = cpool.tile([128, D], bf16, tag="wo")
            bias_s = cpool.tile([128, 8], f32, tag="biases")
            kt_s = cpool.tile([128, LK], bf16, tag="kt")
            qt_s = cpool.tile([128, LQ], bf16, tag="qt")
            v_s = cpool.tile([128, LK], bf16, tag="v")
            ones_s = cpool.tile([128, 1], bf16, tag="ones")
            rs_s = cpool.tile([1, LQ], f32, tag="rs")
            masks_s = cpool.tile([128, 4 * 512], f32, tag="masks")
            ot_s = cpool.tile([128, LQ], bf16, tag="ot")
            identb_s = cpool.tile([128, 128], bf16, tag="identb")

            # first xt block + wk first so PE can start ASAP; xt1 ahead of
            # wv/wq so rb1's K can follow rb0 without a DMA underrun; wo is
            # deferred (not needed until the first out-projection).
            xts = {}
            nc.sync.dma_start(wk_s[:], wk_d.ap())
            nc.sync.dma_start(bias_s[:], bias_d.ap())
            xts[0] = xtpool.tile([128, ND, 512], bf16, tag="xt", name="xt")
            for ch in range(4):
                nc.sync.dma_start(
                    xts[0][:, ch * 4:(ch + 1) * 4, :],
                    xr_d.ap()[:, 0, ch * 4:(ch + 1) * 4, :],
                )
            nc.sync.dma_start(wv_s[:], wv_d.ap())
            nc.sync.dma_start(wq_s[:], wq_d.ap())
            xts[1] = xtpool.tile([128, ND, 512], bf16, tag="xt", name="xt")
            nc.sync.dma_start(xts[1][:], xr_d.ap()[:, 1])

            make_identity(nc, identb_s[:])
            nc.gpsimd.memset(ones_s[:], 1.0)
            # 4 causal mask tiles for relative offsets delta = 0,128,256,384:
            # keep 0 where q_free >= k_part + delta, else MASKVAL
            nc.gpsimd.memset(masks_s[:], 0.0)
            for m in range(4):
                nc.gpsimd.affine_select(
                    out=masks_s[:, m * 512:(m + 1) * 512],
                    in_=masks_s[:, m * 512:(m + 1) * 512],
                    compare_op=mybir.AluOpType.is_ge,
                    fill=MASKVAL,
                    base=-(m * 128),
                    channel_multiplier=-1,
                    pattern=[[1, 512]],
                )

            bq_ap = bias_s[:, 0:1]
            bk_ap = bias_s[:, 1:2]
            bv_ap = bias_s[:, 2:3]
            slot_bias = {"bA": bias_s[:, 3:4], "bB": bias_s[:, 4:5]}

            def prefetch(rb):
                xts[rb] = xtpool.tile([128, ND, 512], bf16, tag="xt",
                                      name="xt")
                nc.sync.dma_start(xts[rb][:], xr_d.ap()[:, rb])

            def emit_rb_gen(rb, prefetch_rb=None):
                """Projections for one 512-wide column block of xr.
                Yields between ~1us chunks so it can fill attention gaps."""
                xt = xts.pop(rb)
                if prefetch_rb is not None:
                    prefetch(prefetch_rb)
                cs = slice(rb * 512, (rb + 1) * 512)

                pk = psum.tile([128, 512], f32, tag="acc512", bufs=2,
                               name="pk")
                for dt in range(ND):
                    nc.tensor.matmul(
                        pk[:], wk_s[:, dt, :], xt[:, dt, :],
                        start=(dt == 0), stop=(dt == ND - 1),
                    )
                    if dt % 4 == 3:
                        yield
                nc.vector.tensor_scalar_add(kt_s[:, cs], pk[:], bk_ap)

                pv = psum.tile([128, 512], f32, tag="acc512", bufs=2,
                               name="pv")
                for dt in range(ND):
                    nc.tensor.matmul(
                        pv[:], wv_s[:, dt, :], xt[:, dt, :],
                        start=(dt == 0), stop=(dt == ND - 1),
                    )
                    if dt % 4 == 3:
                        yield
                vt_tmp = vtpool.tile([128, 512], bf16, tag="vt_tmp")
                nc.vector.tensor_scalar_add(vt_tmp[:], pv[:], bv_ap)
                for s in range(4):
                    ktile = rb * 4 + s
                    vp = psum.tile([128, 128], bf16, tag="acc512", bufs=2,
                                   name="vp")
                    nc.tensor.transpose(
                        vp[:], vt_tmp[:, s * 128:(s + 1) * 128], identb_s[:]
                    )
                    nc.vector.tensor_copy(
                        v_s[:, ktile * 128:(ktile + 1) * 128], vp[:]
                    )
                yield

                if rb < LQ // 512:
                    pq = psum.tile([128, 512], f32, tag="acc512", bufs=2,
                                   name="pq")
                    for dt in range(ND):
                        nc.tensor.matmul(
                            pq[:], wq_s[:, dt, :], xt[:, dt, :],
                            start=(dt == 0), stop=(dt == ND - 1),
                        )
                        if dt % 4 == 3:
                            yield
                    nc.vector.tensor_scalar_add(qt_s[:, cs], pq[:], bq_ap)

            def emit_rb(rb, prefetch_rb=None):
                for _ in emit_rb_gen(rb, prefetch_rb):
                    pass

            def build_pairs(phase, u):
                """Pairs of k-tiles sharing one exp: (kt_a, kt_b, mask_off,
                bkey). mask_off indexes masks_s[:, off:off+1024]."""
                pairs = []
                for kblk, kind in SLOTS[phase]:
                    tiles = []
                    for t in range(8):
                        if kind == "diag":
                            drel = t * 128 - u * 512
                            if drel >= 512:
                                continue
                            midx = drel // 128 if drel >= 0 else None
                            tiles.append((kblk * 8 + t, midx))
                        else:
                            tiles.append((kblk * 8 + t, None))
                    bkey = kind if kind in slot_bias else None
                    # tiles with masks come in runs of consecutive midx
                    i = 0
                    while i < len(tiles):
                        (ta, ma), (tb, mb) = tiles[i], tiles[i + 1]
                        assert (ma is None) == (mb is None)
                        moff = None if ma is None else ma * 512
                        pairs.append((ta, tb, moff, bkey))
                        i += 2
                return pairs

            def emit_attn_u(phase, u, filler=None, nfill=1):
                q0 = phase * BLK + u * 512
                pairs = build_pairs(phase, u)
                n = len(pairs)
                ngroups = n // 2
                ot_acc = psum.tile([128, 512], f32, tag="otacc", bufs=1,
                                   name="ot_acc")
                rs_acc = psum.tile([1, 512], f32, tag="rs", bufs=1,
                                   name="rs_acc")
                ests = [None] * n

                def emit_pair(pi):
                    ta, tb, moff, bkey = pairs[pi]
                    stp = psum.tile([128, 1024], f32, tag="stp", bufs=2,
                                    name="stp")
                    nc.tensor.matmul(
                        stp[:, 0:512],
                        kt_s[:, ta * 128:(ta + 1) * 128],
                        qt_s[:, q0:q0 + 512],
                        start=True, stop=True,
                    )
                    nc.tensor.matmul(
                        stp[:, 512:1024],
                        kt_s[:, tb * 128:(tb + 1) * 128],
                        qt_s[:, q0:q0 + 512],
                        start=True, stop=True,
                    )
                    if moff is not None:
                        nc.vector.tensor_add(
                            stp[:], stp[:], masks_s[:, moff:moff + 1024]
                        )
                    est = epool.tile([128, 1024], bf16, tag="est")
                    nc.scalar.activation(
                        est[:], stp[:],
                        mybir.ActivationFunctionType.Exp,
                        bias=slot_bias[bkey] if bkey else 0.0,
                    )
                    ests[pi] = est

                emit_pair(0)
                if n > 1:
                    emit_pair(1)
                for pi in range(n):
                    ta, tb, moff, bkey = pairs[pi]
                    if pi + 2 < n:
                        emit_pair(pi + 2)
                    if filler is not None:
                        for _ in range(nfill):
                            next(filler, None)
                    est = ests[pi]
                    nc.tensor.matmul(
                        ot_acc[:],
                        v_s[:, ta * 128:(ta + 1) * 128],
                        est[:, 0:512],
                        start=(pi == 0), stop=False,
                    )
                    nc.tensor.matmul(
                        ot_acc[:],
                        v_s[:, tb * 128:(tb + 1) * 128],
                        est[:, 512:1024],
                        start=False, stop=(pi == n - 1),
                    )
                    if pi % 2 == 1:
                        g = pi // 2
                        esum = epool.tile([128, 1024], bf16, tag="esum",
                                          name="esum")
                        nc.vector.tensor_add(
                            esum[:], ests[pi - 1][:], est[:]
                        )
                        fold = epool.tile([128, 512], bf16, tag="fold",
                                          name="fold")
                        nc.vector.tensor_add(
                            fold[:], esum[:, 0:512], esum[:, 512:1024]
                        )
                        nc.tensor.matmul(
                            rs_acc[:], ones_s[:], fold[:],
                            start=(g == 0), stop=(g == ngroups - 1),
                        )

                qb = phase * 2 + u
                nc.vector.tensor_copy(
                    ot_s[:, qb * 512:(qb + 1) * 512], ot_acc[:]
                )
                nc.vector.tensor_copy(
                    rs_s[:, qb * 512:(qb + 1) * 512], rs_acc[:]
                )
                if filler is not None:
                    for _ in filler:  # drain unconsumed filler chunks
                        pass

            def outproj_gen(qb, on_act=False, nstores=2):
                """Out-projection for one 512-query block into a bf16 slab.
                Stores go on the gpsimd SWDGE queue so they never wait
                behind input-prefetch WAR stalls on the sync queue. Yields
                per dt chunk. on_act alternates copies onto ACT (only for
                regions where ACT is not running exp)."""
                slab = outpool.tile([128, ND, 512], bf16, tag="oslab",
                                    name="oslab")
                per = ND // nstores
                for dt in range(ND):
                    po = psum.tile([128, 512], f32, tag="acc512", bufs=2,
                                   name="po")
                    nc.tensor.matmul(
                        po[:],
                        wo_s[:, dt * 128:(dt + 1) * 128],
                        ot_s[:, qb * 512:(qb + 1) * 512],
                        start=True, stop=True,
                    )
                    if on_act and dt % 2 == 1:
                        nc.scalar.activation(
                            slab[:, dt, :], po[:],
                            mybir.ActivationFunctionType.Copy,
                        )
                    else:
                        nc.vector.tensor_copy(slab[:, dt, :], po[:])
                    if dt % per == per - 1:
                        s = dt + 1 - per
                        nc.gpsimd.dma_start(
                            out_d.ap()[:, qb, s:dt + 1], slab[:, s:dt + 1]
                        )
                    yield

            def emit_outproj(qb, on_act=False, nstores=2):
                for _ in outproj_gen(qb, on_act, nstores):
                    pass

            def chain(*gens):
                for g in gens:
                    for x in g:
                        yield x

            # ---- interleaved schedule ----
            # phase 0 needs local kblks 0 (rbs 0,1) and 2 (rbs 4,5) plus
            # Qt[0:1024) (rbs 0,1); phase 1 needs everything.
            emit_rb(0, prefetch_rb=4)
            nc.sync.dma_start(wo_s[:], wo_d.ap())
            emit_rb(1, prefetch_rb=5)
            emit_rb(4, prefetch_rb=2)
            emit_rb(5, prefetch_rb=3)
            emit_attn_u(0, 0, filler=emit_rb_gen(2, prefetch_rb=6))
            emit_attn_u(0, 1, filler=emit_rb_gen(3, prefetch_rb=7))
            emit_rb(6)
            emit_rb(7)
            emit_attn_u(1, 0, filler=chain(outproj_gen(0), outproj_gen(1)),
                        nfill=2)
            emit_attn_u(1, 1, filler=outproj_gen(2), nfill=2)
            emit_outproj(3, on_act=True, nstores=4)
            nc.sync.dma_start(rs_d.ap(), rs_s[:])

    nc.compile()
    return nc


def _get_program():
    if "nc" not in _cached:
        _cached["nc"] = _build_program()
    return _cached["nc"]


def _perm_blocks(i):
    # local order [qA, qB, o1, o2]
    return [0, 3, 1, 2] if i == 0 else [1, 2, 0, 3]


def _repack_w(w):
    # (D, HD) -> [128, ND, 128] with per-partition contiguous lines
    return np.ascontiguousarray(
        w.reshape(ND, 128, HD).transpose(1, 0, 2)
    ).astype(ml_dtypes.bfloat16)


def make_in_maps(x, Wq, bq, Wk, bk, Wv, bv, Wo, bo):
    scale = 1.0 / np.sqrt(np.float32(HD))
    wq_r = _repack_w((Wq * scale).astype(np.float32))
    wk_r = _repack_w(Wk.astype(np.float32))
    wv_r = _repack_w(Wv.astype(np.float32))
    bq_s = (bq * scale).astype(np.float32)
    in_maps = []
    for c in range(8):
        i, b = c % 2, c // 2
        perm = _perm_blocks(i)
        xbT = x[b].T  # (D, L) view
        xT = np.concatenate(
            [xbT[:, p * BLK:(p + 1) * BLK] for p in perm], axis=1
        )
        # (D, L) -> [128, NRB, ND, 512]: xr[p, rb, dt, c] = xT[dt*128+p,
        # rb*512+c]
        xr = np.ascontiguousarray(
            xT.reshape(ND, 128, NRB, 512).transpose(1, 2, 0, 3)
        ).astype(ml_dtypes.bfloat16)
        biases = np.zeros((128, 8), np.float32)
        biases[:, 0] = bq_s
        biases[:, 1] = bk.astype(np.float32)
        biases[:, 2] = bv.astype(np.float32)
        biases[:, 3] = NEG if i == 0 else 0.0   # phase A, slot kblk=2
        biases[:, 4] = 0.0 if i == 0 else NEG   # phase B, slot kblk=3
        in_maps.append({
            "xr": xr,
            "wq": wq_r,
            "wk": wk_r,
            "wv": wv_r,
            "wo": Wo.astype(ml_dtypes.bfloat16),
            "biases": biases,
        })
    return in_maps


def assemble_output(results, bo):
    out = np.empty((B, L, D), np.float32)
    for c in range(8):
        i, b = c % 2, c // 2
        perm = _perm_blocks(i)
        arr = np.asarray(results[c]["outT"], dtype=np.float32)
        # [128, 4, ND, 512] -> (D, LQ)
        outT = arr.transpose(2, 0, 1, 3).reshape(D, LQ)
        outT /= np.asarray(results[c]["rowsums"], dtype=np.float32)
        qA, qB = perm[0], perm[1]
        out[b, qA * BLK:(qA + 1) * BLK, :] = outT[:, 0:BLK].T
        out[b, qB * BLK:(qB + 1) * BLK, :] = outT[:, BLK:2 * BLK].T
    out += bo.astype(np.float32)
    return out


def kernel(x, Wq, bq, Wk, bk, Wv, bv, Wo, bo):
    from concourse.bass_utils import run_bass_kernel_spmd

    nc = _get_program()
    in_maps = make_in_maps(
        np.asarray(x), np.asarray(Wq), np.asarray(bq), np.asarray(Wk),
        np.asarray(bk), np.asarray(Wv), np.asarray(bv), np.asarray(Wo),
        np.asarray(bo),
    )
    res = run_bass_kernel_spmd(nc, in_maps, core_ids=list(range(8)))
    return assemble_output(res.results, np.asarray(bo))
